# revision 10
# baseline (speedup 1.0000x reference)
"""Trainium2 Bass kernel for nn_CharDistributionAnalyzer.

Per-row char histogram features over x:[B=262144, L=128] int32 tokens in [0, 40),
token 0 = padding. Output [B, 6] fp32:
  [unique/40, max_freq, min_freq(masked), letter_ratio, digit_ratio, special_ratio]

Strategy (pure data-parallel over 8 cores, 32768 rows each):
  - Tokens-transposed layout Xt[128 tok, rows] bf16 per 2048-row super-block.
  - Per vocab value v=1..39: equality mask on DVE/GPSIMD (bf16, 4x mode on DVE).
  - PE reduces masks over the token (partition) axis via ones-column stationaries,
    col-tiled into 3 concurrent 32-column groups; psum[slot, row] accumulates the
    39 per-bin counts plus linear features (letter/digit/special/total) for free
    through extra stationary columns.
  - PE transposes counts back to rows-on-partitions; small DVE reduces produce
    max/min/unique; scalar math assembles the 6 features.
"""

import os
import numpy as np

import concourse.bass as bass
import concourse.bacc as bacc
import concourse.mybir as mybir
from concourse.tile import TileContext
from concourse import masks as cmasks
from concourse.bass_utils import run_bass_kernel_spmd

N_CORES = 8
B_FULL = 262144
L = 128
V = 40
R_CORE = B_FULL // N_CORES  # 32768 rows per core

SB = 1024                  # rows per super-block
NBLK = SB // 128           # 16 token-transpose blocks per super-block
NBANK = SB // 512          # 4 psum banks per super-block

# stream v=1..39 -> col group g(v) = (v-1) % 3, slot s(v) = (v-1)//3 in [0,13)
# within-group stationary cols: 0..12 = count slots, 13 letter, 14 digit,
# 15 special, 16 total. psum partition = 32*g + col.
W_COLS = 32
N_GROUPS = 3
S_LET, S_DIG, S_SPE, S_TOT = 13, 14, 15, 16

AF = mybir.ActivationFunctionType
ALU = mybir.AluOpType
DT = mybir.dt
AX = mybir.AxisListType


def build_bass(rows=R_CORE, dve_bins=None):
    """Build the per-core Bass module. `rows` must be a multiple of SB."""
    assert rows % SB == 0
    nsb = rows // SB
    if dve_bins is None:
        dve_bins = set(range(1, 40))  # all masks on DVE (GPSIMD is ~45x slower here)

    nc = bacc.Bacc("TRN2")
    x = nc.dram_tensor("x", [rows, L], DT.int32, kind="ExternalInput")
    wall_d = nc.dram_tensor("wall", [128, 39 * W_COLS], DT.bfloat16, kind="ExternalInput")
    perm_d = nc.dram_tensor("perm", [96, 51], DT.bfloat16, kind="ExternalInput")
    ident_d = nc.dram_tensor("identbf", [128, 128], DT.bfloat16, kind="ExternalInput")
    out = nc.dram_tensor("out", [rows, 6], DT.float32, kind="ExternalOutput")

    with TileContext(nc) as tc:
        with (
            tc.tile_pool(name="const", bufs=1) as constp,
            tc.tile_pool(name="xraw", bufs=2) as xrawp,
            tc.tile_pool(name="xbf", bufs=2) as xbfp,
            tc.tile_pool(name="xt", bufs=2) as xtp,
            tc.tile_pool(name="mask", bufs=4) as maskp,
            tc.tile_pool(name="csb", bufs=2) as csbp,
            tc.tile_pool(name="small", bufs=2) as smallp,
            tc.tile_pool(name="feat", bufs=2) as featp,
            tc.tile_pool(name="psum_c", bufs=2, space="PSUM") as psum_c,
            tc.tile_pool(name="psum_t", bufs=2, space="PSUM") as psum_t,
        ):
            # ---- constants (DMA'd from host-built inputs) ----
            w_all = constp.tile([128, 39 * W_COLS], DT.bfloat16)
            nc.sync.dma_start(out=w_all[:], in_=wall_d[:, :])
            perm = constp.tile([96, 51], DT.bfloat16)
            nc.sync.dma_start(out=perm[:], in_=perm_d[:, :])
            identbf = constp.tile([128, 128], DT.bfloat16)
            nc.sync.dma_start(out=identbf[:], in_=ident_d[:, :])

            for i in range(nsb):
                # ---- load + convert + transpose ----
                x_rows = x[i * SB : (i + 1) * SB, :].rearrange(
                    "(p j) l -> p j l", p=128
                )  # row = i*SB + p*NBLK + j
                xraw = xrawp.tile([128, NBLK, L], DT.int32)
                nc.sync.dma_start(out=xraw[:], in_=x_rows)

                xbf = xbfp.tile([128, NBLK, L], DT.bfloat16)
                nc.scalar.copy(out=xbf[:], in_=xraw[:])

                xt = xtp.tile([128, NBLK, 128], DT.bfloat16)  # [tok, blk, rowpos]
                for j in range(NBLK):
                    nc.sync.dma_start_transpose(out=xt[:, j, :], in_=xbf[:, j, :])

                xt2d = xt[:].rearrange("t j r -> t (j r)")  # [128, SB]

                # ---- masks + PE accumulate ----
                counts = psum_c.tile([128, NBANK, 512], DT.float32)
                for v in range(1, V):
                    mask = maskp.tile([128, SB], DT.bfloat16, tag="mask")
                    eng = nc.vector if v in dve_bins else nc.gpsimd
                    eng.tensor_scalar(
                        out=mask[:], in0=xt2d, scalar1=float(v), scalar2=None,
                        op0=ALU.is_equal,
                    )
                    g = (v - 1) % 3
                    w_v = w_all[:, (v - 1) * W_COLS : v * W_COLS]
                    first = (v - 1) // 3 == 0  # first stream of this group
                    last = (v - 1) // 3 == 12  # last stream of this group
                    for b in range(NBANK):
                        nc.tensor.matmul(
                            counts[32 * g : 32 * g + W_COLS, b, :],
                            w_v,
                            mask[:, b * 512 : (b + 1) * 512],
                            start=first,
                            stop=last,
                            skip_group_check=True,
                        )

                # ---- counts -> SBUF -> transpose+permute back to rows layout ----
                csb = csbp.tile([96, NBANK * 512], DT.bfloat16)
                nc.scalar.copy(out=csb[:], in_=counts[0:96].rearrange("p b f -> p (b f)"))

                # tr[rowpos, blk, d] = csb[perm_src(d), blk*128+rowpos]
                tr = psum_t.tile([128, NBLK, 64], DT.float32)
                for j in range(NBLK):
                    nc.tensor.matmul(
                        tr[:, j, 0:51],
                        csb[:, j * 128 : (j + 1) * 128],
                        perm[:],
                        start=True,
                        stop=True,
                    )

                cnt_grid = tr[:, :, 0:39]          # count of v = d+1
                let_grid = tr[:, :, 39:42]
                dig_grid = tr[:, :, 42:45]
                spe_grid = tr[:, :, 45:48]
                tot_grid = tr[:, :, 48:51]

                # ---- nonlinear features ----
                posc = smallp.tile([128, NBLK, 39], DT.bfloat16, tag="posc")
                nc.vector.tensor_scalar(
                    out=posc[:], in0=cnt_grid, scalar1=0.5, scalar2=None,
                    op0=ALU.is_lt,
                )  # 1.0 where count == 0
                mmin = smallp.tile([128, NBLK, 39], DT.float32, tag="mmin")
                nc.vector.scalar_tensor_tensor(
                    out=mmin[:], in0=posc[:], scalar=1024.0, in1=cnt_grid,
                    op0=ALU.mult, op1=ALU.add,
                )  # count + 1024*(count==0)

                maxc = smallp.tile([128, NBLK], DT.float32, tag="maxc")
                nc.vector.tensor_reduce(out=maxc[:], in_=cnt_grid, axis=AX.X, op=ALU.max)
                minc = smallp.tile([128, NBLK], DT.float32, tag="minc")
                nc.vector.tensor_reduce(out=minc[:], in_=mmin[:], axis=AX.X, op=ALU.min)
                sposc = smallp.tile([128, NBLK], DT.float32, tag="sposc")
                nc.vector.tensor_reduce(out=sposc[:], in_=posc[:], axis=AX.X, op=ALU.add)

                letc = smallp.tile([128, NBLK], DT.float32, tag="letc")
                nc.vector.tensor_reduce(out=letc[:], in_=let_grid, axis=AX.X, op=ALU.add)
                digc = smallp.tile([128, NBLK], DT.float32, tag="digc")
                nc.vector.tensor_reduce(out=digc[:], in_=dig_grid, axis=AX.X, op=ALU.add)
                spec = smallp.tile([128, NBLK], DT.float32, tag="spec")
                nc.vector.tensor_reduce(out=spec[:], in_=spe_grid, axis=AX.X, op=ALU.add)
                tot = smallp.tile([128, NBLK], DT.float32, tag="tot")
                nc.vector.tensor_reduce(out=tot[:], in_=tot_grid, axis=AX.X, op=ALU.add)

                gate = smallp.tile([128, NBLK], DT.float32, tag="gate")
                nc.vector.tensor_scalar(
                    out=gate[:], in0=tot[:], scalar1=0.5, scalar2=None, op0=ALU.is_gt
                )
                tc_ = smallp.tile([128, NBLK], DT.float32, tag="tc")
                nc.vector.tensor_scalar(
                    out=tc_[:], in0=tot[:], scalar1=1.0, scalar2=None, op0=ALU.max
                )
                invt = smallp.tile([128, NBLK], DT.float32, tag="invt")
                nc.vector.reciprocal(out=invt[:], in_=tc_[:])

                feat = featp.tile([128, NBLK, 6], DT.float32)
                # unique = (39 - sum(posc)) / 40
                nc.vector.tensor_scalar(
                    out=feat[:, :, 0], in0=sposc[:], scalar1=-1.0 / 40.0,
                    scalar2=39.0 / 40.0, op0=ALU.mult, op1=ALU.add,
                )
                nc.vector.tensor_tensor(
                    out=feat[:, :, 1], in0=maxc[:], in1=invt[:], op=ALU.mult
                )
                tmp = smallp.tile([128, NBLK], DT.float32, tag="tmp")
                nc.vector.tensor_tensor(
                    out=tmp[:], in0=minc[:], in1=invt[:], op=ALU.mult
                )
                nc.vector.tensor_tensor(
                    out=feat[:, :, 2], in0=tmp[:], in1=gate[:], op=ALU.mult
                )
                nc.vector.tensor_tensor(
                    out=feat[:, :, 3], in0=letc[:], in1=invt[:], op=ALU.mult
                )
                nc.vector.tensor_tensor(
                    out=feat[:, :, 4], in0=digc[:], in1=invt[:], op=ALU.mult
                )
                nc.vector.tensor_tensor(
                    out=feat[:, :, 5], in0=spec[:], in1=invt[:], op=ALU.mult
                )

                out_rows = out[i * SB : (i + 1) * SB, :].rearrange(
                    "(p j) f -> p j f", p=128
                )
                nc.sync.dma_start(out=out_rows, in_=feat[:])

    nc.compile()
    return nc


def build_wall():
    w = np.zeros((128, 39 * W_COLS), np.float32)
    for v in range(1, V):
        base = (v - 1) * W_COLS
        cols = [(v - 1) // 3, S_TOT]
        if 1 <= v <= 26:
            cols.append(S_LET)
        elif 27 <= v <= 36:
            cols.append(S_DIG)
        else:
            cols.append(S_SPE)
        for c in cols:
            w[:, base + c] = 1.0
    import ml_dtypes
    return w.astype(ml_dtypes.bfloat16)


def build_identbf():
    import ml_dtypes
    return np.eye(128, dtype=np.float32).astype(ml_dtypes.bfloat16)


def build_perm():
    p = np.zeros((96, 51), np.float32)
    for d in range(51):
        if d < 39:
            g, sl = d % 3, d // 3
        else:
            g = (d - 39) % 3
            sl = S_LET + (d - 39) // 3
        p[32 * g + sl, d] = 1.0
    import ml_dtypes
    return p.astype(ml_dtypes.bfloat16)


_NC_CACHE = {}


def _get_nc():
    if "nc" not in _NC_CACHE:
        _NC_CACHE["nc"] = build_bass()
    return _NC_CACHE["nc"]


def kernel(x: np.ndarray) -> np.ndarray:
    x = np.asarray(x, dtype=np.int32)
    assert x.shape == (B_FULL, L), x.shape
    nc = _get_nc()
    wall, perm, identbf = build_wall(), build_perm(), build_identbf()
    in_maps = [
        {
            "x": np.ascontiguousarray(x[c * R_CORE : (c + 1) * R_CORE]),
            "wall": wall,
            "perm": perm,
            "identbf": identbf,
        }
        for c in range(N_CORES)
    ]
    res = run_bass_kernel_spmd(nc, in_maps, core_ids=list(range(N_CORES)))
    return np.concatenate([res.results[c]["out"] for c in range(N_CORES)], axis=0)
